# revision 1
# baseline (speedup 1.0000x reference)
"""Trainium2 Bass kernel for nn_MiMoMoeAttention — v2.

Tensor-parallel over heads across 8 NeuronCores (4 q heads + 1 kv head
per core); o_proj row-sharded with partial sums reduced on the host.

Restructuring vs the original baseline (585us -> 414us):
- ACT (scalar engine) runs Exp (single table load, batched 2-head
  instructions with 3D trimmed access patterns) plus half the output
  PSUM->SBUF casts; QKV bias-adds and the other casts on DVE.
- Softmax denominator: DVE accumulates Esum in fp16 (sink/128 folded
  into the init), one all-ones matmul per head broadcasts the
  partition sum, reciprocal via the single-op DVE approx (~18 bits).
  Eliminates 432 per-block denominator matmuls and all ACT table
  thrash.
- o_proj of chunk c-1 is interleaved into B-phase + attention of
  chunk c (paced emission) so the PE never idles while ACT runs exp;
  chunk 0's attention pre-runs half of chunk 1's q-projection instead.
- Explicit 4-tag PSUM bank map (2 banks each) reused across phases;
  V transposed on the PE into a rotation tile at attention start.
- All weights (incl. Wo) resident in SBUF; Wo no longer re-streamed
  (saves 12MB of HBM reads per core).
- fp16 partial outputs (halves write traffic); host reduces in f32.
- 12 junk warmup matmuls keep the PE HAM clock-gate open across the
  startup DMA window; startup loads split so kt 0 lands early.
"""
import numpy as np
from contextlib import ExitStack

from concourse import bacc
import concourse.tile as tile
import concourse.mybir as mybir
from concourse.alu_op_type import AluOpType
from concourse.bass_utils import run_bass_kernel_spmd

dt = mybir.dt
AF = mybir.ActivationFunctionType

B, S, HID = 1, 2048, 4096
H, HK, D = 32, 8, 128
WIN = 1024
THETA = 1000000.0
NCORES = 8
HQ = H // NCORES            # 4 query heads per core
CH = 512                    # token chunk width
NCH = S // CH               # 4 chunks
KT = HID // 128             # 32 contraction tiles
NE = HID // CH              # 8 o_proj column chunks
CBLK = CH // 128            # 4 query blocks per chunk
WBLK = WIN // 128           # 8 blocks lookback
SCALE = float(D) ** -0.5


def _build():
    nc = bacc.Bacc("TRN2", target_bir_lowering=False, debug=False,
                   num_devices=NCORES)
    f32, f16 = dt.float32, dt.float16
    # hsT tiled: row block (c*2+half) holds [p, ktl, col] for kt=half*16+ktl
    hsT = nc.dram_tensor("hsT", [NCH * 2 * 128, 16 * CH], f16,
                         kind="ExternalInput").ap()
    # fused qkv weights: [128, KT*768]; per kt: 4x128 q | 128 k | 128 v
    wqkv = nc.dram_tensor("wqkv", [128, KT * 768], f16,
                          kind="ExternalInput").ap()
    # wo: [128, HQ*NE*CH]; slice (jt, e) at (jt*NE+e)*CH
    wo = nc.dram_tensor("wo", [128, HQ * NE * CH], f16,
                        kind="ExternalInput").ap()
    bq = nc.dram_tensor("bq", [128, HQ], f32, kind="ExternalInput").ap()
    bk = nc.dram_tensor("bk", [128, 1], f32, kind="ExternalInput").ap()
    bv = nc.dram_tensor("bv", [128, 1], f32, kind="ExternalInput").ap()
    cosd = nc.dram_tensor("cosd", [128, S], f16, kind="ExternalInput").ap()
    sind = nc.dram_tensor("sind", [128, S], f32, kind="ExternalInput").ap()
    m0 = nc.dram_tensor("m0", [128, 128], f16, kind="ExternalInput").ap()
    m8 = nc.dram_tensor("m8", [128, 128], f16, kind="ExternalInput").ap()
    es4 = nc.dram_tensor("es4", [128, HQ * CH], f16,
                         kind="ExternalInput").ap()
    ident = nc.dram_tensor("ident", [128, 128], f32,
                           kind="ExternalInput").ap()
    pswap = nc.dram_tensor("pswap", [128, 128], f16,
                           kind="ExternalInput").ap()
    allon = nc.dram_tensor("allon", [128, 128], f16,
                           kind="ExternalInput").ap()
    out = nc.dram_tensor("o_part", [S, HID], f16, kind="ExternalOutput").ap()

    with tile.TileContext(nc) as tc, ExitStack() as ctx:
        const = ctx.enter_context(tc.tile_pool(name="const", bufs=1))
        keep = ctx.enter_context(tc.tile_pool(name="keep", bufs=1))
        work = ctx.enter_context(tc.tile_pool(name="work", bufs=1))
        ps = ctx.enter_context(tc.tile_pool(name="ps", bufs=1, space="PSUM"))

        # ---- constant / weight preload (order = DMA queue order) -------
        wqkv_sb = const.tile([128, KT * 768], f16, tag="wqkv", name="wqkv_sb")
        hst_c0 = work.tile([128, 16 * CH], f16, tag="hst", bufs=2,
                           name="hst_c0")
        # interleave first-chunk loads so kt 0 is ready fast
        nc.sync.dma_start(wqkv_sb[:, 0:4 * 768], wqkv[:, 0:4 * 768])
        nc.sync.dma_start(hst_c0[:, 0:4 * CH], hsT[0:128, 0:4 * CH])
        nc.sync.dma_start(wqkv_sb[:, 4 * 768:8 * 768],
                          wqkv[:, 4 * 768:8 * 768])
        nc.sync.dma_start(hst_c0[:, 4 * CH:8 * CH],
                          hsT[0:128, 4 * CH:8 * CH])
        nc.sync.dma_start(wqkv_sb[:, 8 * 768:16 * 768],
                          wqkv[:, 8 * 768:16 * 768])
        nc.sync.dma_start(hst_c0[:, 8 * CH:16 * CH],
                          hsT[0:128, 8 * CH:16 * CH])
        # second half of chunk 0's activations BEFORE the big consts, so
        # the QKV pipeline never starves at startup
        hst_c0b = work.tile([128, 16 * CH], f16, tag="hst", bufs=2,
                            name="hst_c0b")
        nc.sync.dma_start(hst_c0b[:, 0:8 * CH], hsT[128:256, 0:8 * CH])
        nc.sync.dma_start(wqkv_sb[:, 16 * 768:KT * 768],
                          wqkv[:, 16 * 768:KT * 768])
        nc.sync.dma_start(hst_c0b[:, 8 * CH:16 * CH],
                          hsT[128:256, 8 * CH:16 * CH])

        bq_sb = const.tile([128, HQ], f32, tag="bq", name="bq_sb")
        nc.sync.dma_start(bq_sb[:], bq)
        bk_sb = const.tile([128, 1], f32, tag="bk", name="bk_sb")
        nc.sync.dma_start(bk_sb[:], bk)
        bv_sb = const.tile([128, 1], f32, tag="bv", name="bv_sb")
        nc.sync.dma_start(bv_sb[:], bv)
        m0_sb = const.tile([128, 128], f16, tag="m0", name="m0_sb")
        nc.sync.dma_start(m0_sb[:], m0)
        m8_sb = const.tile([128, 128], f16, tag="m8", name="m8_sb")
        nc.sync.dma_start(m8_sb[:], m8)
        id_sb = const.tile([128, 128], f32, tag="ident", name="id_sb")
        nc.sync.dma_start(id_sb[:], ident)
        pw_sb = const.tile([128, 128], f16, tag="pswap", name="pw_sb")
        nc.sync.dma_start(pw_sb[:], pswap)
        ao_sb = const.tile([128, 128], f16, tag="allon", name="ao_sb")
        nc.sync.dma_start(ao_sb[:], allon)
        es_sb = const.tile([128, HQ * CH], f16, tag="es4", name="es_sb")
        nc.sync.dma_start(es_sb[:], es4)
        cos_sb = const.tile([128, S], f16, tag="cos", name="cos_sb")
        nc.sync.dma_start(cos_sb[:], cosd)
        sin_sb = const.tile([128, S], f32, tag="sin", name="sin_sb")
        nc.sync.dma_start(sin_sb[:], sind)
        wo_sb = const.tile([128, HQ * NE * CH], f16, tag="wo", name="wo_sb")
        nc.sync.dma_start(wo_sb[:], wo)

        # persistent rotated K (d-major) and V (t-major per block)
        krotT = keep.tile([128, S], f16, tag="krotT", name="krotT")
        v_all = keep.tile([128, S], f16, tag="v_all", name="v_all")

        # ---- HAM warmup: junk matmuls while the startup DMAs land -----
        junk = const.tile([128, CH], f16, tag="junk", name="junk")
        nc.vector.memset(junk[:], 0.0)
        wps = ps.tile([128, 1024], f32, tag="x", name="wps")
        for _ in range(16):
            nc.tensor.matmul(wps[:, 0:CH], junk[:, 0:128], junk[:],
                             start=True, stop=True)

        m0b = m0_sb[:].unsqueeze(1).broadcast_to([128, 2, 128])
        m8b = m8_sb[:].unsqueeze(1).broadcast_to([128, 2, 128])

        def rope(dst, src_sb, swap_ps, s0):
            t1 = work.tile([128, CH], f16, tag="r1", bufs=2, name="t1")
            nc.vector.tensor_tensor(t1[:], src_sb[:], cos_sb[:, s0:s0 + CH],
                                    op=AluOpType.mult)
            t2 = work.tile([128, CH], f16, tag="r2", bufs=2, name="t2")
            nc.vector.tensor_tensor(t2[:], swap_ps[:], sin_sb[:, s0:s0 + CH],
                                    op=AluOpType.mult)
            nc.vector.tensor_tensor(dst, t1[:], t2[:], op=AluOpType.add)

        # ---------------- o_proj emission (interleaved) -----------------
        class OProj:
            def __init__(self):
                self.an_prev = None
                self.c_prev = -1
                self.done = 0
                self.tags = ("q01",)
                self.tiles = {}
                self.tail = False

            def begin(self, c_prev, an_prev, tags=("q01",), tail=False):
                self.an_prev = an_prev
                self.c_prev = c_prev
                self.done = 0
                self.tags = tags
                self.tiles = {}
                self.tail = tail

            def emit(self, n):
                if self.an_prev is None:
                    return
                n = min(n, NE * CBLK - self.done)
                for _ in range(n):
                    i = self.done
                    e, sb = i // CBLK, i % CBLK
                    ti = (i // 2) % len(self.tags)
                    if ti not in self.tiles:
                        self.tiles[ti] = ps.tile([128, 1024], f32,
                                                 tag=self.tags[ti],
                                                 name="o_t")
                    dst = self.tiles[ti][:, (i % 2) * CH:(i % 2) * CH + CH]
                    for jt in range(HQ):
                        nc.tensor.matmul(
                            dst,
                            self.an_prev[jt][:, sb * 128:(sb + 1) * 128],
                            wo_sb[:, (jt * NE + e) * CH:(jt * NE + e + 1) * CH],
                            start=jt == 0, stop=jt == HQ - 1)
                    osb = work.tile([128, CH], f16, tag="osb", bufs=8,
                                    name="osb")
                    r0 = self.c_prev * CH + sb * 128
                    if i % 2 == 0:
                        nc.scalar.copy(osb[:], dst)
                        nc.sync.dma_start(
                            out[r0:r0 + 128, e * CH:(e + 1) * CH], osb[:])
                    else:
                        nc.vector.tensor_copy(osb[:], dst)
                        # ACT queue is exp-free in the tail: use its hwdge
                        (nc.scalar if self.tail else nc.sync).dma_start(
                            out[r0:r0 + 128, e * CH:(e + 1) * CH], osb[:])
                    self.done += 1

            def flush(self):
                self.emit(NE * CBLK)

        oproj = OProj()

        lg_rot = [0]

        def attn_pass(heads, qrot, blo, bhi, n_o_slots, filler=None):
            """Sliding-window attention for a pair of heads."""
            oproj.emit(1)  # keep PE fed across the pass boundary
            atp = ps.tile([128, 1024], f32, tag="q23", name="atp")
            nc.vector.memset(atp[:], 0.0)
            esum = work.tile([128, 1024], f16, tag="esum", name="esum")
            p0 = heads[0] * CH
            nc.gpsimd.tensor_copy(esum[:], es_sb[:, p0:p0 + 2 * CH])
            esum3 = esum[:].rearrange("p (h w) -> p h w", h=2)

            jlist = list(range(max(0, blo - WBLK), bhi + 1))
            pend = None
            slot = 0
            for j in jlist:
                lo, hi = max(j, blo), min(j + WBLK, bhi)
                c0 = (lo - blo) * 128
                w = (hi - lo + 1) * 128
                lg = ps.tile([128, 1024], f32,
                             tag=("x", "kv")[lg_rot[0] % 2], name="lg")
                lg_rot[0] += 1
                for ih in range(2):
                    nc.tensor.matmul(lg[:, ih * CH + c0:ih * CH + c0 + w],
                                     krotT[:, j * 128:(j + 1) * 128],
                                     qrot[heads[ih]][:, c0:c0 + w],
                                     start=True, stop=True)
                # keep PE fed while ACT runs exp
                tgt = (slot + 1) * (NE * CBLK) // max(n_o_slots, 1)
                oproj.emit(tgt - oproj.done)
                if filler is not None:
                    filler(slot)
                slot += 1
                E = work.tile([128, 1024], f16, tag="E", bufs=3, name="E")
                lg3 = lg[:].rearrange("p (h w) -> p h w", h=2)[:, :,
                                                              c0:c0 + w]
                E3 = E[:].rearrange("p (h w) -> p h w", h=2)[:, :, c0:c0 + w]
                nc.scalar.activation(E3, lg3, AF.Exp, scale=SCALE)
                E3f = E[:].rearrange("p (h w) -> p h w", h=2)
                if lo == j:  # diagonal block: causal mask (keep r <= c)
                    nc.vector.tensor_tensor(E3f[:, :, c0:c0 + 128],
                                            E3f[:, :, c0:c0 + 128], m0b,
                                            op=AluOpType.mult)
                if hi == j + WBLK:  # far edge: keep r > c
                    nc.vector.tensor_tensor(
                        E3f[:, :, c0 + w - 128:c0 + w],
                        E3f[:, :, c0 + w - 128:c0 + w], m8b,
                        op=AluOpType.mult)
                nc.vector.tensor_tensor(esum3[:, :, c0:c0 + w],
                                        esum3[:, :, c0:c0 + w], E3,
                                        op=AluOpType.add)
                if pend is not None:
                    Ep, c0p, wp, jp = pend
                    for ih in range(2):
                        nc.tensor.matmul(
                            atp[:, ih * CH + c0p:ih * CH + c0p + wp],
                            v_all[:, jp * 128:(jp + 1) * 128],
                            Ep[:, ih * CH + c0p:ih * CH + c0p + wp],
                            start=False, stop=True)
                pend = (E, c0, w, j)
            Ep, c0p, wp, jp = pend
            for ih in range(2):
                nc.tensor.matmul(atp[:, ih * CH + c0p:ih * CH + c0p + wp],
                                 v_all[:, jp * 128:(jp + 1) * 128],
                                 Ep[:, ih * CH + c0p:ih * CH + c0p + wp],
                                 start=False, stop=True)
            # denominator: broadcast partition-sum of esum, then 1/x
            rb = ps.tile([128, 1024], f32, tag="x", name="rb")
            lg_rot[0] = 1
            for ih in range(2):
                nc.tensor.matmul(rb[:, ih * CH:(ih + 1) * CH], ao_sb[:],
                                 esum[:, ih * CH:(ih + 1) * CH],
                                 start=True, stop=True)
            # 1/x: single custom-DVE op, ~18 correct bits (enough for the
            # fp16 an); InstReciprocal costs 6.5us and ACT Ln thrashes the
            # activation table set.
            rcp = work.tile([128, 1024], f32, tag="rcp", bufs=2, name="rcp")
            nc.vector.reciprocal_approx_fast(rcp[:], rb[:])
            an_o = {}
            for ih, h in enumerate(heads):
                an = work.tile([128, CH], f16, tag=f"an{h}", bufs=2,
                               name="an")
                nc.vector.tensor_tensor(an[:], atp[:, ih * CH:(ih + 1) * CH],
                                        rcp[:, ih * CH:(ih + 1) * CH],
                                        op=AluOpType.mult)
                an_o[h] = an
            return an_o

        prev_an = None
        hst_next = {0: hst_c0, 1: hst_c0b}
        for c in range(NCH):
            s0 = c * CH
            # ---- A: fused q/k/v projection --------------------------
            if c == 1:
                q01 = saved_q01["t"]   # pre-filled during chunk 0's attn
                jts = range(2, HQ)
            else:
                q01 = ps.tile([128, 1024], f32, tag="q01", name="q01")
                jts = range(HQ)
            q23 = ps.tile([128, 1024], f32, tag="q23", name="q23")
            kvp = ps.tile([128, 1024], f32, tag="kv", name="kvp")
            qps = (q01, q23)
            for half in range(2):
                hst = hst_next[half]
                for ktl in range(16):
                    kt = half * 16 + ktl
                    w0 = kt * 768
                    first, last = kt == 0, kt == KT - 1
                    rhs = hst[:, ktl * CH:(ktl + 1) * CH]
                    for jt in jts:
                        nc.tensor.matmul(
                            qps[jt // 2][:, (jt % 2) * CH:(jt % 2 + 1) * CH],
                            wqkv_sb[:, w0 + jt * 128:w0 + (jt + 1) * 128],
                            rhs, start=first, stop=last)
                    nc.tensor.matmul(kvp[:, 0:CH],
                                     wqkv_sb[:, w0 + 512:w0 + 640], rhs,
                                     start=first, stop=last)
                    nc.tensor.matmul(kvp[:, CH:2 * CH],
                                     wqkv_sb[:, w0 + 640:w0 + 768], rhs,
                                     start=first, stop=last)

            # ---- B: bias/rope/transpose post-processing -------------
            oproj.begin(c - 1, prev_an)
            kT = work.tile([128, CH], f16, tag="kT", name="kT")
            nc.vector.tensor_scalar_add(kT[:], kvp[:, 0:CH], bk_sb[:, 0:1])
            qT = {}
            for jt in range(2):
                qt = work.tile([128, CH], f16, tag=f"qT{jt}", name="qt")
                nc.vector.tensor_scalar_add(
                    qt[:], qps[0][:, jt * CH:(jt + 1) * CH],
                    bq_sb[:, jt:jt + 1])
                qT[jt] = qt
            vT = work.tile([128, CH], f32, tag="vT", name="vT")
            nc.vector.tensor_scalar_add(vT[:], kvp[:, CH:2 * CH],
                                        bv_sb[:, 0:1])
            for jt in range(2, HQ):
                qt = work.tile([128, CH], f16, tag=f"qT{jt}", name="qt")
                nc.vector.tensor_scalar_add(
                    qt[:], qps[1][:, (jt - 2) * CH:(jt - 1) * CH],
                    bq_sb[:, jt:jt + 1])
                qT[jt] = qt
            # rope swaps (PE) into x / kv psum tiles; qsw3 reuses the
            # ksw slot after the k-rope read (subtile WAR dep)
            x_t = ps.tile([128, 1024], f32, tag="x", name="x_t")
            sw_t = ps.tile([128, 1024], f32, tag="kv", name="sw_t")
            nc.tensor.matmul(x_t[:, 0:CH], pw_sb[:], kT[:],
                             start=True, stop=True)
            rope(krotT[:, s0:s0 + CH], kT, x_t[:, 0:CH], s0)
            oproj.emit(1)
            qsw = [x_t[:, CH:2 * CH], sw_t[:, 0:CH], sw_t[:, CH:2 * CH],
                   x_t[:, 0:CH]]
            qrot = {}
            for jt in range(HQ):
                nc.tensor.matmul(qsw[jt], pw_sb[:], qT[jt][:],
                                 start=True, stop=True)
                qr = work.tile([128, CH], f16, tag=f"qr{jt}", name="qr")
                rope(qr[:], qT[jt], qsw[jt], s0)
                qrot[jt] = qr
                oproj.emit(1)

            # prefetch next chunk's activations before the out-DMA queue
            # fills up
            if c + 1 < NCH:
                for half in range(2):
                    t = work.tile([128, 16 * CH], f16, tag="hst", bufs=2,
                                  name="hst_pf")
                    nc.sync.dma_start(
                        t[:], hsT[((c + 1) * 2 + half) * 128:
                                  ((c + 1) * 2 + half + 1) * 128])
                    hst_next[half] = t

            # chunk 0 has no o_proj to interleave: pre-run QKV(1)'s
            # q01-head matmuls in its attention instead
            filler = None
            if c == 0:
                q01_c1 = ps.tile([128, 1024], f32, tag="q01", name="q01_c1")
                fst = {"kt": 0}

                def filler(slot, n_slots=8):
                    tgt_kt = min(KT, (slot + 1) * KT // n_slots)
                    while fst["kt"] < tgt_kt:
                        kt = fst["kt"]
                        w0 = kt * 768
                        rhs1 = hst_next[kt // 16][:, (kt % 16) * CH:
                                                  (kt % 16 + 1) * CH]
                        for jt in range(2):
                            nc.tensor.matmul(
                                q01_c1[:, jt * CH:(jt + 1) * CH],
                                wqkv_sb[:, w0 + jt * 128:w0 + (jt + 1) * 128],
                                rhs1, start=kt == 0, stop=kt == KT - 1)
                        fst["kt"] += 1
                saved_q01 = {"t": q01_c1}

            # ---- attention for this chunk + o_proj of previous ------
            # V transpose on the PE into a rotation psum tile; v_all(c)
            # blocks are first consumed ~8 j-iterations into pass 1
            vtt = ps.tile([128, 1024], f32,
                          tag=("x", "kv")[lg_rot[0] % 2], name="vtt")
            lg_rot[0] += 1
            for i in range(CBLK):
                nc.tensor.transpose(vtt[:, i * 128:(i + 1) * 128],
                                    vT[:, i * 128:(i + 1) * 128], id_sb[:])
                nc.vector.tensor_copy(
                    v_all[:, s0 + i * 128:s0 + (i + 1) * 128],
                    vtt[:, i * 128:(i + 1) * 128])
            blo, bhi = c * CBLK, c * CBLK + CBLK - 1
            n_j = bhi - max(0, blo - WBLK) + 1
            an_all = {}
            an_all.update(attn_pass((0, 1), qrot, blo, bhi, 2 * n_j,
                                    filler))
            an_all.update(attn_pass((2, 3), qrot, blo, bhi, 2 * n_j,
                                    filler))
            oproj.flush()
            if c == 0:
                while fst["kt"] < KT:
                    filler(99, 100)
            prev_an = [an_all[h] for h in range(HQ)]

        oproj.begin(NCH - 1, prev_an, tags=("q01", "x", "kv", "q23"),
                    tail=True)
        oproj.flush()

    nc.compile()
    return nc


_CACHED = None
_LAST_IN_MAPS = None


def _get_nc():
    global _CACHED
    if _CACHED is None:
        _CACHED = _build()
    return _CACHED


def kernel(positions, hidden_states, Wq, bq, Wk, bk, Wv, bv, Wo, sink,
           **_ignored):
    positions = np.asarray(positions)
    hidden_states = np.asarray(hidden_states, dtype=np.float32)
    Wq = np.asarray(Wq, dtype=np.float32)
    Wk = np.asarray(Wk, dtype=np.float32)
    Wv = np.asarray(Wv, dtype=np.float32)
    Wo = np.asarray(Wo, dtype=np.float32)
    bq = np.asarray(bq, dtype=np.float32)
    bk = np.asarray(bk, dtype=np.float32)
    bv = np.asarray(bv, dtype=np.float32)
    sink = np.asarray(sink, dtype=np.float32)

    # host-derived tables
    half = D // 2
    inv_freq = 1.0 / (THETA ** (np.arange(half, dtype=np.float64) / half))
    ang = positions[0].astype(np.float64)[None, :] * inv_freq[:, None]
    cos64 = np.cos(ang).astype(np.float16)
    sin64 = np.sin(ang).astype(np.float32)
    cosd = np.ascontiguousarray(np.concatenate([cos64, cos64], axis=0))
    sind = np.ascontiguousarray(np.concatenate([-sin64, sin64], axis=0))
    r, cidx = np.arange(128)[:, None], np.arange(128)[None, :]
    m0 = (r <= cidx).astype(np.float16)
    m8 = (r > cidx).astype(np.float16)
    ident = np.eye(128, dtype=np.float32)
    pswap = np.zeros((128, 128), dtype=np.float16)
    pswap[np.arange(128), (np.arange(128) + 64) % 128] = 1.0
    allon = np.ones((128, 128), dtype=np.float16)

    # hsT: [HID, S] -> [c, half, p, ktl, col] -> [NCH*2*128, 16*CH]
    hsT_full = np.ascontiguousarray(hidden_states[0].T).astype(np.float16)
    hsT_t = np.ascontiguousarray(
        hsT_full.reshape(2, 16, 128, NCH, CH).transpose(3, 0, 2, 1, 4)
        .reshape(NCH * 2 * 128, 16 * CH))
    esink_all = np.exp(sink.astype(np.float64)).astype(np.float64)

    in_maps = []
    for core in range(NCORES):
        qs = slice(core * HQ * D, (core + 1) * HQ * D)
        ks = slice(core * D, (core + 1) * D)
        wq_c = Wq[:, qs].astype(np.float16).reshape(KT, 128, HQ * D)
        wk_c = Wk[:, ks].astype(np.float16).reshape(KT, 128, D)
        wv_c = Wv[:, ks].astype(np.float16).reshape(KT, 128, D)
        wqkv_c = np.concatenate([wq_c, wk_c, wv_c], axis=2)  # [KT,128,768]
        wqkv_t = np.ascontiguousarray(
            wqkv_c.transpose(1, 0, 2).reshape(128, KT * 768))
        wo_t = np.ascontiguousarray(
            Wo[qs, :].astype(np.float16).reshape(HQ, 128, NE, CH)
            .transpose(1, 0, 2, 3).reshape(128, HQ * NE * CH))
        es4 = np.zeros((128, HQ * CH), dtype=np.float16)
        for h in range(HQ):
            es4[:, h * CH:(h + 1) * CH] = np.float16(
                esink_all[core * HQ + h] / 128.0)
        in_maps.append(dict(
            hsT=hsT_t, wqkv=wqkv_t, wo=wo_t,
            bq=np.ascontiguousarray(bq[qs].reshape(HQ, D).T),
            bk=np.ascontiguousarray(bk[ks].reshape(D, 1)),
            bv=np.ascontiguousarray(bv[ks].reshape(D, 1)),
            cosd=cosd, sind=sind, m0=m0, m8=m8, es4=es4,
            ident=ident, pswap=pswap, allon=allon,
        ))

    global _LAST_IN_MAPS
    _LAST_IN_MAPS = in_maps
    nc = _get_nc()
    res = None
    for attempt in range(3):
        try:
            res = run_bass_kernel_spmd(nc, in_maps, list(range(NCORES)))
            break
        except Exception:
            if attempt == 2:
                raise
            import time as _t
            _t.sleep(2.0)
    out = np.zeros((S, HID), dtype=np.float32)
    for core in range(NCORES):
        out += res.results[core]["o_part"].astype(np.float32)
    return out.reshape(B, S, HID)



# revision 3
# speedup vs baseline: 1.3037x; 1.3037x over previous
"""Trainium2 Bass kernel for nn_MiMoMoeAttention — v3.

Tensor-parallel over heads across 8 NeuronCores (4 q heads + 1 kv head
per core); o_proj row-sharded with partial sums reduced on the host.

v3 restructuring vs v2 (421us):
- PSUM remapped from four 2-bank tiles to eight 1-bank tags (p0-p7).
  Tile-granular WAR tracking made v2's single-tile o_proj serialize at
  ~2.2us/emission (mm group -> psum copy -> next mm group); with p6/p7
  ping-ponging as o_proj destinations the next emission's matmuls no
  longer wait on the previous emission's PSUM->SBUF cast.
- o_proj pacing now uses a chunk-global slot counter; v2's per-pass
  target formula never advanced past 16/32 emissions in pass 2, piling
  ~12-16 emissions into a fully serialized flush at each chunk edge
  (~25us lost in the tail alone).
- Per-head logits tiles (p2-p5 rotation) and per-head PV accumulators
  (p0/p1); denominator rb tiles pinned to p4/p5 so the next chunk's
  k/v chains only WAR the fast rcp reads.
- Tail o_proj rotates 4 psum tags and alternates sync/scalar DMA
  queues, running at matmul rate instead of copy-chain rate.
"""
import numpy as np
from contextlib import ExitStack

from concourse import bacc
import concourse.tile as tile
import concourse.mybir as mybir
from concourse.alu_op_type import AluOpType
from concourse.bass_utils import run_bass_kernel_spmd

dt = mybir.dt
AF = mybir.ActivationFunctionType

B, S, HID = 1, 2048, 4096
H, HK, D = 32, 8, 128
WIN = 1024
THETA = 1000000.0
NCORES = 8
HQ = H // NCORES            # 4 query heads per core
CH = 512                    # token chunk width
NCH = S // CH               # 4 chunks
KT = HID // 128             # 32 contraction tiles
NE = HID // CH              # 8 o_proj column chunks
CBLK = CH // 128            # 4 query blocks per chunk
WBLK = WIN // 128           # 8 blocks lookback
SCALE = float(D) ** -0.5


def _build():
    nc = bacc.Bacc("TRN2", target_bir_lowering=False, debug=False,
                   num_devices=NCORES)
    f32, f16 = dt.float32, dt.float16
    # hsT tiled: row block (c*2+half) holds [p, ktl, col] for kt=half*16+ktl
    hsT = nc.dram_tensor("hsT", [NCH * 2 * 128, 16 * CH], f16,
                         kind="ExternalInput").ap()
    # fused qkv weights: [128, KT*768]; per kt: 4x128 q | 128 k | 128 v
    wqkv = nc.dram_tensor("wqkv", [128, KT * 768], f16,
                          kind="ExternalInput").ap()
    # wo: [128, HQ*NE*CH]; slice (jt, e) at (jt*NE+e)*CH
    wo = nc.dram_tensor("wo", [128, HQ * NE * CH], f16,
                        kind="ExternalInput").ap()
    bq = nc.dram_tensor("bq", [128, HQ], f32, kind="ExternalInput").ap()
    bk = nc.dram_tensor("bk", [128, 1], f32, kind="ExternalInput").ap()
    bv = nc.dram_tensor("bv", [128, 1], f32, kind="ExternalInput").ap()
    cosd = nc.dram_tensor("cosd", [128, S], f16, kind="ExternalInput").ap()
    sind = nc.dram_tensor("sind", [128, S], f32, kind="ExternalInput").ap()
    m0 = nc.dram_tensor("m0", [128, 128], f16, kind="ExternalInput").ap()
    m8 = nc.dram_tensor("m8", [128, 128], f16, kind="ExternalInput").ap()
    es4 = nc.dram_tensor("es4", [128, HQ * CH], f16,
                         kind="ExternalInput").ap()
    ident = nc.dram_tensor("ident", [128, 128], f32,
                           kind="ExternalInput").ap()
    pswap = nc.dram_tensor("pswap", [128, 128], f16,
                           kind="ExternalInput").ap()
    allon = nc.dram_tensor("allon", [128, 128], f16,
                           kind="ExternalInput").ap()
    out = nc.dram_tensor("o_part", [S, HID], f16, kind="ExternalOutput").ap()

    with tile.TileContext(nc) as tc, ExitStack() as ctx:
        const = ctx.enter_context(tc.tile_pool(name="const", bufs=1))
        keep = ctx.enter_context(tc.tile_pool(name="keep", bufs=1))
        work = ctx.enter_context(tc.tile_pool(name="work", bufs=1))
        ps = ctx.enter_context(tc.tile_pool(name="ps", bufs=1, space="PSUM"))

        def pst(tag):
            return ps.tile([128, CH], f32, tag=tag, name=f"ps_{tag}")

        # ---- constant / weight preload (order = DMA queue order) -------
        wqkv_sb = const.tile([128, KT * 768], f16, tag="wqkv", name="wqkv_sb")
        hst_c0 = work.tile([128, 16 * CH], f16, tag="hst", bufs=2,
                           name="hst_c0")
        # interleave first-chunk loads so kt 0 is ready fast
        nc.sync.dma_start(wqkv_sb[:, 0:4 * 768], wqkv[:, 0:4 * 768])
        nc.sync.dma_start(hst_c0[:, 0:4 * CH], hsT[0:128, 0:4 * CH])
        nc.sync.dma_start(wqkv_sb[:, 4 * 768:8 * 768],
                          wqkv[:, 4 * 768:8 * 768])
        nc.sync.dma_start(hst_c0[:, 4 * CH:8 * CH],
                          hsT[0:128, 4 * CH:8 * CH])
        nc.sync.dma_start(wqkv_sb[:, 8 * 768:16 * 768],
                          wqkv[:, 8 * 768:16 * 768])
        nc.sync.dma_start(hst_c0[:, 8 * CH:16 * CH],
                          hsT[0:128, 8 * CH:16 * CH])
        # second half of chunk 0's activations BEFORE the big consts, so
        # the QKV pipeline never starves at startup
        hst_c0b = work.tile([128, 16 * CH], f16, tag="hst", bufs=2,
                            name="hst_c0b")
        nc.sync.dma_start(hst_c0b[:, 0:8 * CH], hsT[128:256, 0:8 * CH])
        nc.sync.dma_start(wqkv_sb[:, 16 * 768:KT * 768],
                          wqkv[:, 16 * 768:KT * 768])
        nc.sync.dma_start(hst_c0b[:, 8 * CH:16 * CH],
                          hsT[128:256, 8 * CH:16 * CH])

        bq_sb = const.tile([128, HQ], f32, tag="bq", name="bq_sb")
        nc.sync.dma_start(bq_sb[:], bq)
        bk_sb = const.tile([128, 1], f32, tag="bk", name="bk_sb")
        nc.sync.dma_start(bk_sb[:], bk)
        bv_sb = const.tile([128, 1], f32, tag="bv", name="bv_sb")
        nc.sync.dma_start(bv_sb[:], bv)
        m0_sb = const.tile([128, 128], f16, tag="m0", name="m0_sb")
        nc.sync.dma_start(m0_sb[:], m0)
        m8_sb = const.tile([128, 128], f16, tag="m8", name="m8_sb")
        nc.sync.dma_start(m8_sb[:], m8)
        id_sb = const.tile([128, 128], f32, tag="ident", name="id_sb")
        nc.sync.dma_start(id_sb[:], ident)
        pw_sb = const.tile([128, 128], f16, tag="pswap", name="pw_sb")
        nc.sync.dma_start(pw_sb[:], pswap)
        ao_sb = const.tile([128, 128], f16, tag="allon", name="ao_sb")
        nc.sync.dma_start(ao_sb[:], allon)
        es_sb = const.tile([128, HQ * CH], f16, tag="es4", name="es_sb")
        nc.sync.dma_start(es_sb[:], es4)
        cos_sb = const.tile([128, S], f16, tag="cos", name="cos_sb")
        nc.sync.dma_start(cos_sb[:], cosd)
        sin_sb = const.tile([128, S], f32, tag="sin", name="sin_sb")
        nc.sync.dma_start(sin_sb[:], sind)
        wo_sb = const.tile([128, HQ * NE * CH], f16, tag="wo", name="wo_sb")
        nc.sync.dma_start(wo_sb[:], wo)

        # persistent rotated K (d-major) and V (t-major per block)
        krotT = keep.tile([128, S], f16, tag="krotT", name="krotT")
        v_all = keep.tile([128, S], f16, tag="v_all", name="v_all")

        # ---- HAM warmup: junk matmuls while the startup DMAs land -----
        junk = const.tile([128, CH], f16, tag="junk", name="junk")
        nc.vector.memset(junk[:], 0.0)
        wps = pst("p6")
        for _ in range(8):
            nc.tensor.matmul(wps[:], junk[:, 0:128], junk[:],
                             start=True, stop=True)
        wps2 = pst("p7")
        for _ in range(8):
            nc.tensor.matmul(wps2[:], junk[:, 0:128], junk[:],
                             start=True, stop=True)

        m0b = m0_sb[:].unsqueeze(1).broadcast_to([128, 2, 128])
        m8b = m8_sb[:].unsqueeze(1).broadcast_to([128, 2, 128])

        def rope(dst, src_sb, swap_ps, s0):
            t1 = work.tile([128, CH], f16, tag="r1", bufs=2, name="t1")
            nc.vector.tensor_tensor(t1[:], src_sb[:], cos_sb[:, s0:s0 + CH],
                                    op=AluOpType.mult)
            t2 = work.tile([128, CH], f16, tag="r2", bufs=2, name="t2")
            nc.vector.tensor_tensor(t2[:], swap_ps[:], sin_sb[:, s0:s0 + CH],
                                    op=AluOpType.mult)
            nc.vector.tensor_tensor(dst, t1[:], t2[:], op=AluOpType.add)

        # ---------------- o_proj emission (interleaved) -----------------
        class OProj:
            def __init__(self):
                self.an_prev = None
                self.c_prev = -1
                self.done = 0
                self.rot = ("p6", "p7")
                self.tail = False

            def begin(self, c_prev, an_prev, rot=("p6", "p7"), tail=False):
                self.an_prev = an_prev
                self.c_prev = c_prev
                self.done = 0
                self.rot = rot
                self.tail = tail

            def emit(self, n):
                if self.an_prev is None:
                    return
                n = min(n, NE * CBLK - self.done)
                for _ in range(n):
                    i = self.done
                    e, sb = i // CBLK, i % CBLK
                    dst = pst(self.rot[i % len(self.rot)])
                    for jt in range(HQ):
                        nc.tensor.matmul(
                            dst[:],
                            self.an_prev[jt][:, sb * 128:(sb + 1) * 128],
                            wo_sb[:, (jt * NE + e) * CH:(jt * NE + e + 1) * CH],
                            start=jt == 0, stop=jt == HQ - 1)
                    osb = work.tile([128, CH], f16, tag="osb", bufs=8,
                                    name="osb")
                    r0 = self.c_prev * CH + sb * 128
                    if i % 2 == 0:
                        nc.scalar.copy(osb[:], dst[:])
                        nc.sync.dma_start(
                            out[r0:r0 + 128, e * CH:(e + 1) * CH], osb[:])
                    else:
                        nc.vector.tensor_copy(osb[:], dst[:])
                        # ACT queue is exp-free in the tail: use its hwdge
                        (nc.scalar if self.tail else nc.sync).dma_start(
                            out[r0:r0 + 128, e * CH:(e + 1) * CH], osb[:])
                    self.done += 1

            def flush(self):
                self.emit(NE * CBLK)

        oproj = OProj()
        lg_rot = ["p2", "p3", "p4", "p5"]

        def attn_pass(heads, qrot, blo, bhi, slot0, n_slots, filler=None):
            """Sliding-window attention for a pair of heads."""
            oproj.emit(1)  # keep PE fed across the pass boundary
            atp = [pst("p0"), pst("p1")]
            nc.vector.memset(atp[0][:], 0.0)
            nc.vector.memset(atp[1][:], 0.0)
            esum = work.tile([128, 1024], f16, tag="esum", bufs=2,
                             name="esum")
            p0 = heads[0] * CH
            nc.gpsimd.tensor_copy(esum[:], es_sb[:, p0:p0 + 2 * CH])
            esum3 = esum[:].rearrange("p (h w) -> p h w", h=2)

            jlist = list(range(max(0, blo - WBLK), bhi + 1))
            pend = None
            slot = 0
            rk = [0]
            for j in jlist:
                lo, hi = max(j, blo), min(j + WBLK, bhi)
                c0 = (lo - blo) * 128
                w = (hi - lo + 1) * 128
                lg = [None, None]
                for ih in range(2):
                    lg[ih] = pst(lg_rot[rk[0] % 4])
                    rk[0] += 1
                    nc.tensor.matmul(lg[ih][:, c0:c0 + w],
                                     krotT[:, j * 128:(j + 1) * 128],
                                     qrot[heads[ih]][:, c0:c0 + w],
                                     start=True, stop=True)
                # keep PE fed while ACT runs exp
                tgt = (slot0 + slot + 1) * (NE * CBLK) // max(n_slots, 1)
                oproj.emit(tgt - oproj.done)
                if filler is not None:
                    filler(slot0 + slot)
                slot += 1
                E = work.tile([128, 1024], f16, tag="E", bufs=3, name="E")
                for ih in range(2):
                    nc.scalar.activation(E[:, ih * CH + c0:ih * CH + c0 + w],
                                         lg[ih][:, c0:c0 + w],
                                         AF.Exp, scale=SCALE)
                E3 = E[:].rearrange("p (h w) -> p h w", h=2)
                E3f = E3
                if lo == j:  # diagonal block: causal mask (keep r <= c)
                    nc.vector.tensor_tensor(E3f[:, :, c0:c0 + 128],
                                            E3f[:, :, c0:c0 + 128], m0b,
                                            op=AluOpType.mult)
                if hi == j + WBLK:  # far edge: keep r > c
                    nc.vector.tensor_tensor(
                        E3f[:, :, c0 + w - 128:c0 + w],
                        E3f[:, :, c0 + w - 128:c0 + w], m8b,
                        op=AluOpType.mult)
                nc.vector.tensor_tensor(esum3[:, :, c0:c0 + w],
                                        esum3[:, :, c0:c0 + w],
                                        E3[:, :, c0:c0 + w],
                                        op=AluOpType.add)
                if pend is not None:
                    Ep, c0p, wp, jp = pend
                    for ih in range(2):
                        nc.tensor.matmul(
                            atp[ih][:, c0p:c0p + wp],
                            v_all[:, jp * 128:(jp + 1) * 128],
                            Ep[:, ih * CH + c0p:ih * CH + c0p + wp],
                            start=False, stop=True)
                pend = (E, c0, w, j)
            Ep, c0p, wp, jp = pend
            for ih in range(2):
                nc.tensor.matmul(atp[ih][:, c0p:c0p + wp],
                                 v_all[:, jp * 128:(jp + 1) * 128],
                                 Ep[:, ih * CH + c0p:ih * CH + c0p + wp],
                                 start=False, stop=True)
            # denominator: broadcast partition-sum of esum, then 1/x.
            # rb pinned to p4/p5: the next consumers are this pass-end rcp
            # reads, which clear before the next chunk's k/v chains.
            rb = [pst("p4"), pst("p5")]
            for ih in range(2):
                nc.tensor.matmul(rb[ih][:], ao_sb[:],
                                 esum[:, ih * CH:(ih + 1) * CH],
                                 start=True, stop=True)
            rcp = work.tile([128, 1024], f32, tag="rcp", bufs=2, name="rcp")
            nc.vector.reciprocal_approx_fast(rcp[:, 0:CH], rb[0][:])
            nc.vector.reciprocal_approx_fast(rcp[:, CH:2 * CH], rb[1][:])
            an_o = {}
            for ih, h in enumerate(heads):
                an = work.tile([128, CH], f16, tag=f"an{h}", bufs=2,
                               name="an")
                nc.vector.tensor_tensor(an[:], atp[ih][:],
                                        rcp[:, ih * CH:(ih + 1) * CH],
                                        op=AluOpType.mult)
                an_o[h] = an
            return an_o

        prev_an = None
        hst_next = {0: hst_c0, 1: hst_c0b}
        saved_q01 = {}
        for c in range(NCH):
            s0 = c * CH
            # ---- A: fused q/k/v projection --------------------------
            if c == 1:
                qps = {0: saved_q01[0], 1: saved_q01[1],
                       2: pst("p0"), 3: pst("p1")}
                jts = range(2, HQ)
            else:
                qps = {jt: pst(f"p{jt}") for jt in range(HQ)}
                jts = range(HQ)
            kps = pst("p4")
            vps = pst("p5")
            for half in range(2):
                hst = hst_next[half]
                for ktl in range(16):
                    kt = half * 16 + ktl
                    w0 = kt * 768
                    first, last = kt == 0, kt == KT - 1
                    rhs = hst[:, ktl * CH:(ktl + 1) * CH]
                    for jt in jts:
                        nc.tensor.matmul(
                            qps[jt][:],
                            wqkv_sb[:, w0 + jt * 128:w0 + (jt + 1) * 128],
                            rhs, start=first, stop=last)
                    nc.tensor.matmul(kps[:],
                                     wqkv_sb[:, w0 + 512:w0 + 640], rhs,
                                     start=first, stop=last)
                    nc.tensor.matmul(vps[:],
                                     wqkv_sb[:, w0 + 640:w0 + 768], rhs,
                                     start=first, stop=last)

            # ---- B: bias/rope/transpose post-processing -------------
            oproj.begin(c - 1, prev_an)
            kT = work.tile([128, CH], f16, tag="kT", name="kT")
            nc.vector.tensor_scalar_add(kT[:], kps[:], bk_sb[:, 0:1])
            qT = {}
            for jt in range(2):
                qt = work.tile([128, CH], f16, tag=f"qT{jt}", name="qt")
                nc.vector.tensor_scalar_add(qt[:], qps[jt][:],
                                            bq_sb[:, jt:jt + 1])
                qT[jt] = qt
            vT = work.tile([128, CH], f32, tag="vT", name="vT")
            nc.vector.tensor_scalar_add(vT[:], vps[:], bv_sb[:, 0:1])
            for jt in range(2, HQ):
                qt = work.tile([128, CH], f16, tag=f"qT{jt}", name="qt")
                nc.vector.tensor_scalar_add(qt[:], qps[jt][:],
                                            bq_sb[:, jt:jt + 1])
                qT[jt] = qt
            # rope swaps (PE) rotate through p4/p5 after the k/v bias
            # reads release them (subtile WAR deps are tile-granular)
            ksw = pst("p5")
            nc.tensor.matmul(ksw[:], pw_sb[:], kT[:], start=True, stop=True)
            rope(krotT[:, s0:s0 + CH], kT, ksw[:], s0)
            oproj.emit(1)
            qrot = {}
            for jt in range(HQ):
                qsw = pst("p4" if jt % 2 == 0 else "p5")
                nc.tensor.matmul(qsw[:], pw_sb[:], qT[jt][:],
                                 start=True, stop=True)
                qr = work.tile([128, CH], f16, tag=f"qr{jt}", name="qr")
                rope(qr[:], qT[jt], qsw[:], s0)
                qrot[jt] = qr
                oproj.emit(1)

            # prefetch next chunk's activations before the out-DMA queue
            # fills up
            if c + 1 < NCH:
                for half in range(2):
                    t = work.tile([128, 16 * CH], f16, tag="hst", bufs=2,
                                  name="hst_pf")
                    nc.sync.dma_start(
                        t[:], hsT[((c + 1) * 2 + half) * 128:
                                  ((c + 1) * 2 + half + 1) * 128])
                    hst_next[half] = t

            # chunk 0 has no o_proj to interleave: pre-run chunk 1's
            # q0/q1 projections into p6/p7 (o_proj's tags, idle in c0)
            filler = None
            if c == 0:
                q01_c1 = {0: pst("p6"), 1: pst("p7")}
                fst = {"kt": 0}

                def filler(slot, n_slots=8):
                    tgt_kt = min(KT, (slot + 1) * KT // n_slots)
                    while fst["kt"] < tgt_kt:
                        kt = fst["kt"]
                        w0 = kt * 768
                        rhs1 = hst_next[kt // 16][:, (kt % 16) * CH:
                                                  (kt % 16 + 1) * CH]
                        for jt in range(2):
                            nc.tensor.matmul(
                                q01_c1[jt][:],
                                wqkv_sb[:, w0 + jt * 128:w0 + (jt + 1) * 128],
                                rhs1, start=kt == 0, stop=kt == KT - 1)
                        fst["kt"] += 1
                saved_q01 = q01_c1

            # ---- attention for this chunk + o_proj of previous ------
            # V transpose on the PE into a rotation psum tile; v_all(c)
            # blocks are first consumed ~8 j-iterations into pass 1
            vtt = pst("p2")
            for i in range(CBLK):
                nc.tensor.transpose(vtt[:, i * 128:(i + 1) * 128],
                                    vT[:, i * 128:(i + 1) * 128], id_sb[:])
                nc.vector.tensor_copy(
                    v_all[:, s0 + i * 128:s0 + (i + 1) * 128],
                    vtt[:, i * 128:(i + 1) * 128])
            blo, bhi = c * CBLK, c * CBLK + CBLK - 1
            n_j = bhi - max(0, blo - WBLK) + 1
            an_all = {}
            an_all.update(attn_pass((0, 1), qrot, blo, bhi, 0, 2 * n_j,
                                    filler))
            an_all.update(attn_pass((2, 3), qrot, blo, bhi, n_j, 2 * n_j,
                                    filler))
            oproj.flush()
            if c == 0:
                while fst["kt"] < KT:
                    filler(99, 100)
            prev_an = [an_all[h] for h in range(HQ)]

        oproj.begin(NCH - 1, prev_an, rot=("p2", "p3", "p6", "p7"),
                    tail=True)
        oproj.flush()

    nc.compile()
    return nc


_CACHED = None
_LAST_IN_MAPS = None


def _get_nc():
    global _CACHED
    if _CACHED is None:
        _CACHED = _build()
    return _CACHED


def kernel(positions, hidden_states, Wq, bq, Wk, bk, Wv, bv, Wo, sink,
           **_ignored):
    positions = np.asarray(positions)
    hidden_states = np.asarray(hidden_states, dtype=np.float32)
    Wq = np.asarray(Wq, dtype=np.float32)
    Wk = np.asarray(Wk, dtype=np.float32)
    Wv = np.asarray(Wv, dtype=np.float32)
    Wo = np.asarray(Wo, dtype=np.float32)
    bq = np.asarray(bq, dtype=np.float32)
    bk = np.asarray(bk, dtype=np.float32)
    bv = np.asarray(bv, dtype=np.float32)
    sink = np.asarray(sink, dtype=np.float32)

    # host-derived tables
    half = D // 2
    inv_freq = 1.0 / (THETA ** (np.arange(half, dtype=np.float64) / half))
    ang = positions[0].astype(np.float64)[None, :] * inv_freq[:, None]
    cos64 = np.cos(ang).astype(np.float16)
    sin64 = np.sin(ang).astype(np.float32)
    cosd = np.ascontiguousarray(np.concatenate([cos64, cos64], axis=0))
    sind = np.ascontiguousarray(np.concatenate([-sin64, sin64], axis=0))
    r, cidx = np.arange(128)[:, None], np.arange(128)[None, :]
    m0 = (r <= cidx).astype(np.float16)
    m8 = (r > cidx).astype(np.float16)
    ident = np.eye(128, dtype=np.float32)
    pswap = np.zeros((128, 128), dtype=np.float16)
    pswap[np.arange(128), (np.arange(128) + 64) % 128] = 1.0
    allon = np.ones((128, 128), dtype=np.float16)

    # hsT: [HID, S] -> [c, half, p, ktl, col] -> [NCH*2*128, 16*CH]
    hsT_full = np.ascontiguousarray(hidden_states[0].T).astype(np.float16)
    hsT_t = np.ascontiguousarray(
        hsT_full.reshape(2, 16, 128, NCH, CH).transpose(3, 0, 2, 1, 4)
        .reshape(NCH * 2 * 128, 16 * CH))
    esink_all = np.exp(sink.astype(np.float64)).astype(np.float64)

    in_maps = []
    for core in range(NCORES):
        qs = slice(core * HQ * D, (core + 1) * HQ * D)
        ks = slice(core * D, (core + 1) * D)
        wq_c = Wq[:, qs].astype(np.float16).reshape(KT, 128, HQ * D)
        wk_c = Wk[:, ks].astype(np.float16).reshape(KT, 128, D)
        wv_c = Wv[:, ks].astype(np.float16).reshape(KT, 128, D)
        wqkv_c = np.concatenate([wq_c, wk_c, wv_c], axis=2)  # [KT,128,768]
        wqkv_t = np.ascontiguousarray(
            wqkv_c.transpose(1, 0, 2).reshape(128, KT * 768))
        wo_t = np.ascontiguousarray(
            Wo[qs, :].astype(np.float16).reshape(HQ, 128, NE, CH)
            .transpose(1, 0, 2, 3).reshape(128, HQ * NE * CH))
        es4 = np.zeros((128, HQ * CH), dtype=np.float16)
        for h in range(HQ):
            es4[:, h * CH:(h + 1) * CH] = np.float16(
                esink_all[core * HQ + h] / 128.0)
        in_maps.append(dict(
            hsT=hsT_t, wqkv=wqkv_t, wo=wo_t,
            bq=np.ascontiguousarray(bq[qs].reshape(HQ, D).T),
            bk=np.ascontiguousarray(bk[ks].reshape(D, 1)),
            bv=np.ascontiguousarray(bv[ks].reshape(D, 1)),
            cosd=cosd, sind=sind, m0=m0, m8=m8, es4=es4,
            ident=ident, pswap=pswap, allon=allon,
        ))

    global _LAST_IN_MAPS
    _LAST_IN_MAPS = in_maps
    nc = _get_nc()
    res = None
    for attempt in range(3):
        try:
            res = run_bass_kernel_spmd(nc, in_maps, list(range(NCORES)))
            break
        except Exception:
            if attempt == 2:
                raise
            import time as _t
            _t.sleep(2.0)
    out = np.zeros((S, HID), dtype=np.float32)
    for core in range(NCORES):
        out += res.results[core]["o_part"].astype(np.float32)
    return out.reshape(B, S, HID)
